# revision 1
# baseline (speedup 1.0000x reference)
"""Sliding-window attention block (B=4, S=2048, E=1024, H=16, D=64,
window_left=512, window_right=0) on 8 Trainium2 NeuronCores.

Sharding: core c handles batch b=c//2 and head group g=c%2 (8 heads each).
Each core computes qkv projection for its heads over the full sequence,
banded attention (256-query stripes, 128-key blocks), and a partial output
projection; the host sums the two head-group partials per batch.

All device dataflow is feature-major (transposed): qkT/attnT/outT are
[features, seq].  Window masking is added into the scores PSUM with
identity-weight matmuls of precomputed -30000 bias tiles.  Key padding
(j >= seq_len) is handled by zeroing V rows and the denominator-ones
column, so padded keys drop out of both numerator and denominator.
Fully-masked query rows (i >= seq_len+512) are fixed up on the host
(reference semantics: uniform attention over all keys).
"""

import numpy as np

B, S, E, H, D = 4, 2048, 1024, 16, 64
NCORES = 8
HPC = H // 2          # heads per core
WIN = 512             # window_left (window_right = 0)
NEG = -30000.0
NQ = 256              # query stripe width
NST = S // NQ         # stripes
SCALE = 1.0 / np.sqrt(np.float32(D))

_cache = {}


def _build_program(repeat=1, ablate=()):
    from contextlib import ExitStack

    import concourse.bass as bass  # noqa: F401
    import concourse.mybir as mybir
    import concourse.tile as tile
    from concourse import bacc

    dt = mybir.dt
    f32, f32r = dt.float32, dt.float32r
    AF = mybir.ActivationFunctionType
    mult = mybir.AluOpType.mult

    nc = bacc.Bacc("TRN2", target_bir_lowering=False, debug=False,
                   num_devices=NCORES)
    bf16 = dt.bfloat16

    xT = nc.dram_tensor("xT", [E, S], bf16, kind="ExternalInput")
    # wqk laid out [partition, k-chunk, out-col] so per-mb column slices
    # stream in 0.25 MB pieces (fast first-matmul arrival)
    wqk = nc.dram_tensor("wqk", [128, 8, 2 * HPC * D], bf16,
                         kind="ExternalInput")
    wv = nc.dram_tensor("wv", [E, HPC * D], bf16, kind="ExternalInput")
    wo = nc.dram_tensor("wo", [HPC * D, E], f32r, kind="ExternalInput")
    vmask = nc.dram_tensor("vmask", [128, 16], f32, kind="ExternalInput")
    vone8 = nc.dram_tensor("vone8", [S, HPC], bf16, kind="ExternalInput")
    # mask planes (cols): [0:512) = [m_a | m_d]; [512:768) =
    # [m_b[:,128:] | m_c[:,:128]]; [768:1152) = [m_d | m_c[:,:128]];
    # [1152:1280) = identity
    # f32r to match the score matmuls' weight dtype — a bf16 128-col ident
    # would toggle the compiler's FWL weight-load mode on every masked bank
    masks = nc.dram_tensor("masks", [128, 1280], f32r, kind="ExternalInput")
    outT = nc.dram_tensor("outT", [E, S], bf16, kind="ExternalOutput")

    with tile.TileContext(nc) as tc:
      for _rep in range(repeat):
       with ExitStack() as ctx:
        persist = ctx.enter_context(tc.tile_pool(name="persist", bufs=1))

        qdt = bf16 if "bfqk" in ablate else f32r
        qkT = [persist.tile([128, S], qdt, name=f"qkT{i}", tag=f"qkT{i}") for i in range(8)]
        vsb = [persist.tile([128, HPC, D + 1], bf16, name=f"v{t}", tag=f"v{t}")
               for t in range(16)]
        mskT = persist.tile([128, 1280], f32r, tag="mskT")
        ident = mskT[:, 1152:1280]
        vmsb = persist.tile([128, 16], f32, tag="vmsb")
        attnT = [persist.tile([128, S], f32r, name=f"attnT{i}",
                              tag=f"attnT{i}") for i in range(4)]
        wo_sb = [persist.tile([128, E], f32r, name=f"wo{c}", tag=f"wo{c}")
                 for c in range(4)]

        # ---- fused schedule: projection chunk nb, then attention stripes
        # 2nb / 2nb+1 (whose keys end at tile 4nb+3); the stripe-pair's
        # output projection is interleaved into chunk nb+1's qk loop.
        with tc.tile_pool(name="wgt12", bufs=1) as wpool, \
             tc.tile_pool(name="xc", bufs=3) as xpool, \
             tc.tile_pool(name="qkps", bufs=2, space="PSUM") as qkps, \
             tc.tile_pool(name="vps", bufs=2, space="PSUM") as vps, \
             tc.tile_pool(name="scps", bufs=2, space="PSUM") as spool, \
             tc.tile_pool(name="ops", bufs=1, space="PSUM") as opool, \
             tc.tile_pool(name="oprj", bufs=1, space="PSUM") as ppool, \
             tc.tile_pool(name="expT", bufs=6) as epool, \
             tc.tile_pool(name="ob", bufs=3) as obpool, \
             tc.tile_pool(name="rc", bufs=4) as rpool, \
             tc.tile_pool(name="rb", bufs=4) as rbpool:
            xcs = {}

            def load_chunk(nb):
                tiles = [xpool.tile([128, 512], bf16, name=f"xc{k}",
                                    tag=f"xc{k}") for k in range(8)]
                for k in range(8):
                    nc.sync.dma_start(
                        out=tiles[k],
                        in_=xT[k * 128:(k + 1) * 128,
                               nb * 512:(nb + 1) * 512])
                xcs[nb] = tiles

            wqk_sb = wpool.tile([128, 8, 2 * HPC * D], bf16, name="wqk",
                                tag="wqk")
            if "nowdma" not in ablate:
                for mb in range(2):
                    nc.sync.dma_start(
                        out=wqk_sb[:, :, mb * 128:(mb + 1) * 128],
                        in_=wqk[:, :, mb * 128:(mb + 1) * 128])
            else:
                nc.vector.memset(wqk_sb[:, 0, 0:8], 0.125)
            load_chunk(0)
            if "nowdma" not in ablate:
                for mb in range(2, 8):
                    nc.sync.dma_start(
                        out=wqk_sb[:, :, mb * 128:(mb + 1) * 128],
                        in_=wqk[:, :, mb * 128:(mb + 1) * 128])
            wv_sb = [wpool.tile([128, HPC * D], bf16, name=f"wv{k}",
                                tag=f"wv{k}") for k in range(8)]
            for k in range(8):
                if "nowdma" not in ablate:
                    nc.sync.dma_start(out=wv_sb[k],
                                      in_=wv[k * 128:(k + 1) * 128, :])
                else:
                    nc.vector.memset(wv_sb[k][:, 0:8], 0.125)
            nc.sync.dma_start(out=mskT, in_=masks[:, :])
            nc.sync.dma_start(out=vmsb, in_=vmask[:, :])
            domask = "mask" not in ablate

            def stripe_banks(s):
                # Banks pair key blocks so every mask-bias region is ONE
                # contiguous >=256-wide slice of the flat [128, 512] PSUM
                # bank.  Mask spec = (flat_lo, flat_hi, msk_col).
                if s == 0:
                    kb_banks = [([5, 4], (0, 384, 768))]
                elif s == 1:
                    kb_banks = [([2, 3], None), ([5, 4], (0, 384, 768))]
                else:
                    kb_banks = [([0, 5], (0, 512, 0)),
                                ([1, 4], (128, 384, 512)),
                                ([2, 3], None)]
                return kb_banks

            def emit_scores(s, hg, kb_banks):
                base_kt = 2 * s - 4
                all_exs = {}
                for bi, (bks, mspec) in enumerate(kb_banks):
                    scbs = {h: spool.tile([128, 2 * NQ], f32, name=f"sc{h}",
                                          tag="sc") for h in hg}
                    for li, kb in enumerate(bks):
                        for h in hg:
                            po = (h % 2) * 64
                            ktile = base_kt + kb
                            nc.tensor.matmul(
                                scbs[h][:, li * NQ:(li + 1) * NQ],
                                lhsT=qkT[4 + h // 2][
                                    po:po + 64,
                                    ktile * 128:(ktile + 1) * 128],
                                rhs=qkT[h // 2][po:po + 64,
                                                s * NQ:(s + 1) * NQ],
                                start=(li == 0),
                                stop=(li == len(bks) - 1
                                      and not (domask and mspec
                                               and "pemask" in ablate)))
                    if domask and mspec:
                        lo, hi, mc = mspec
                        for h in hg:
                            if "pemask" in ablate:
                                nc.tensor.matmul(
                                    scbs[h][:, lo:hi],
                                    lhsT=ident[:, :],
                                    rhs=mskT[:, mc:mc + hi - lo],
                                    start=False, stop=True)
                            else:
                                # mask bias via DVE add into PSUM — identity
                                # matmuls stall the PE weight-load pipeline
                                nc.vector.tensor_tensor(
                                    out=scbs[h][:, lo:hi],
                                    in0=scbs[h][:, lo:hi],
                                    in1=mskT[:, mc:mc + hi - lo],
                                    op=mybir.AluOpType.add)
                    for h in hg:
                        exb = epool.tile([128, 2 * NQ], bf16, tag="ex")
                        nc.scalar.activation(exb[:, :], scbs[h][:, :],
                                             AF.Exp)
                        all_exs.setdefault(h, []).append(exb)
                return all_exs

            def emit_av(s, hg, kb_banks, all_exs):
                base_kt = 2 * s - 4
                kb_pos = {kb: (bi, li)
                          for bi, (bks, _) in enumerate(kb_banks)
                          for li, kb in enumerate(bks)}
                kbs = sorted(kb_pos)
                # one PSUM bank holds both heads' AV accumulations; the
                # first matmul's start clears the whole bank, so the second
                # head's group must NOT re-assert start
                ot2 = opool.tile([D + 1, len(hg), NQ], f32, tag="ot")
                for hi, h in enumerate(hg):
                    exs = all_exs[h]
                    ot = ot2[:, hi, :]
                    for i, kb in enumerate(kbs):
                        bi2, li = kb_pos[kb]
                        ktile = base_kt + kb
                        nc.tensor.matmul(
                            ot[:, :],
                            lhsT=vsb[ktile][:, h, :],
                            rhs=exs[bi2][:, li * NQ:(li + 1) * NQ],
                            start=(i == 0 and hi == 0),
                            stop=(i == len(kbs) - 1))
                nhg = len(hg)
                if "norm" not in ablate:
                    # one reciprocal + one broadcast covers the whole group
                    rc = rpool.tile([1, nhg, NQ], f32, tag="rc")
                    nc.vector.reciprocal(rc[:, :, :], ot2[D:D + 1, :, :])
                    rb = rbpool.tile([128, nhg, NQ], f32, tag="rb")
                    nc.gpsimd.partition_broadcast(
                        rb.rearrange("p a q -> p (a q)"),
                        rc.rearrange("p a q -> p (a q)"))
                for hi, h in enumerate(hg):
                    po = (h % 2) * 64
                    ot = ot2[:, hi, :]
                    dst = attnT[h // 2][po:po + 64, s * NQ:(s + 1) * NQ]
                    if "norm" in ablate:
                        nc.vector.tensor_copy(dst, ot[0:D, :])
                    else:
                        # evict unnormalized, freeing the PSUM slot quickly;
                        # scale attnT in place off-path
                        if hi % 2 == 0:
                            nc.scalar.copy(dst, ot[0:D, :])
                        else:
                            nc.vector.tensor_copy(dst, ot[0:D, :])
                        nc.vector.tensor_tensor(
                            out=dst, in0=dst, in1=rb[po:po + 64, hi, :],
                            op=mult)

            def emit_outproj_mb(c0, mb, final=False):
                if final:
                    # projection psum pools are idle by now — rotate through
                    # them so the ob copy never blocks the next group
                    pool, tag = [(ppool, "pp"), (qkps, "qk"),
                                 (vps, "v")][mb % 3]
                    pp = pool.tile([128, 2 * NQ], f32, tag=tag)
                else:
                    pp = ppool.tile([128, 2 * NQ], f32, tag="pp")
                for cb in range(4):
                    nc.tensor.matmul(
                        pp[:, :],
                        lhsT=wo_sb[cb][:, mb * 128:(mb + 1) * 128],
                        rhs=attnT[cb][:, c0:c0 + 2 * NQ],
                        start=(cb == 0), stop=(cb == 3))
                ob = obpool.tile([128, 2 * NQ], bf16, tag="ob")
                nc.vector.tensor_copy(ob[:, :], pp[:, :])
                if "noout" not in ablate:
                    nc.sync.dma_start(
                        out=outT[mb * 128:(mb + 1) * 128, c0:c0 + 2 * NQ],
                        in_=ob[:, :])

            if "nopair" in ablate:
                head_groups = [(h,) for h in range(HPC)]
            else:
                head_groups = [(2 * i, 2 * i + 1) for i in range(HPC // 2)]
            pending = None  # software-pipeline AV one head-group behind
            out_c0 = None   # outproj deferred into the next chunk's qk loop
            for nb in range(4):
                if "noxdma" in ablate:
                    if 0 not in xcs:
                        load_chunk(0)
                    xc = xcs[0]
                elif nb not in xcs:
                    load_chunk(nb)
                    xc = xcs.pop(nb)
                else:
                    xc = xcs.pop(nb)
                for mb in range(8):
                    ps = qkps.tile([128, 512], f32, tag="qk")
                    for k in range(8):
                        nc.tensor.matmul(
                            ps[:, :],
                            lhsT=wqk_sb[:, k, mb * 128:(mb + 1) * 128],
                            rhs=xc[k][:, :],
                            start=(k == 0), stop=(k == 7))
                    nc.scalar.copy(qkT[mb][:, nb * 512:(nb + 1) * 512],
                                   ps[:, :])
                    # batched so the f32r wo weight loads don't toggle FWL
                    # against the bf16 projection weights on every group
                    if out_c0 is not None and mb in (3, 7):
                        for omb in range(mb - 3, mb + 1):
                            emit_outproj_mb(out_c0, omb)
                out_c0 = None
                if nb + 1 < 4 and "noxdma" not in ablate:
                    load_chunk(nb + 1)
                if nb == 0:
                    for c in range(4):
                        nc.sync.dma_start(out=wo_sb[c],
                                          in_=wo[c * 128:(c + 1) * 128, :])
                for t4 in range(4):
                    t = nb * 4 + t4
                    ps = vps.tile([128, 512], f32, tag="v")
                    for k in range(8):
                        nc.tensor.matmul(
                            ps[:, :],
                            lhsT=xc[k][:, t4 * 128:(t4 + 1) * 128],
                            rhs=wv_sb[k][:, :],
                            start=(k == 0), stop=(k == 7))
                    nc.vector.tensor_scalar(
                        out=vsb[t][:, :, 0:D],
                        in0=ps.rearrange("p (h d) -> p h d", h=HPC),
                        scalar1=vmsb[:, t:t + 1],
                        scalar2=None,
                        op0=mult)
                    nc.sync.dma_start(out=vsb[t][:, :, D],
                                      in_=vone8[t * 128:(t + 1) * 128, :])
                for s in (2 * nb, 2 * nb + 1):
                    kb_banks = stripe_banks(s)
                    for hg in head_groups:
                        all_exs = emit_scores(s, hg, kb_banks)
                        if pending is not None:
                            emit_av(*pending)
                        pending = (s, hg, kb_banks, all_exs)
                    if s % 2 == 1:
                        if pending is not None:
                            emit_av(*pending)
                            pending = None
                        if nb + 1 < 4:
                            out_c0 = (s - 1) * NQ
                        else:
                            for mb in range(8):
                                emit_outproj_mb((s - 1) * NQ, mb, final=True)

    nc.compile()
    return nc


def _prep_inputs(x_padded, Wqkv, Wout, seq_lengths):
    """Per-core input maps."""
    import ml_dtypes
    bf16 = ml_dtypes.bfloat16
    Wq = Wqkv[0:E]
    Wk = Wqkv[E:2 * E]
    Wv = Wqkv[2 * E:3 * E]

    # static window mask tiles (identical for every core)
    p = np.arange(128)[:, None]
    f = np.arange(NQ)[None, :]
    m_a = np.where(f <= p, 0.0, NEG).astype(np.float32)
    m_b = np.where(f <= p + 128, 0.0, NEG).astype(np.float32)
    m_c = np.where(f >= p, 0.0, NEG).astype(np.float32)
    m_d = np.where(f >= p + 128, 0.0, NEG).astype(np.float32)
    masks = np.concatenate([
        m_a, m_d,                      # [0:512)    bank (kb0, kb5)
        m_b[:, 128:], m_c[:, :128],    # [512:768)  bank (kb1, kb4)
        m_d, m_c[:, :128],             # [768:1152) bank (kb5, kb4)
        np.eye(128, dtype=np.float32),  # [1152:1280) identity
    ], axis=1)
    in_maps = []
    for c in range(NCORES):
        b, g = divmod(c, 2)
        hs = np.arange(g * HPC, (g + 1) * HPC)
        rows = (hs[:, None] * D + np.arange(D)[None, :]).reshape(-1)
        wqk_c = np.concatenate([Wq[rows] * SCALE, Wk[rows]], axis=0)
        valid = (np.arange(S) < seq_lengths[b]).astype(np.float32)
        in_maps.append({
            "xT": np.ascontiguousarray(x_padded[b].T).astype(bf16),
            "wqk": np.ascontiguousarray(
                wqk_c.T.reshape(8, 128, 2 * HPC * D).transpose(1, 0, 2)
            ).astype(bf16),
            "wv": np.ascontiguousarray(Wv[rows].T).astype(bf16),
            "wo": np.ascontiguousarray(Wout[:, rows].T),
            "vmask": np.ascontiguousarray(valid.reshape(16, 128).T),
            "vone8": np.ascontiguousarray(
                np.repeat(valid[:, None], HPC, axis=1)).astype(bf16),
            "masks": masks,
        })
    return in_maps


def _make_runner(nc):
    """Reusable jitted SPMD executor (the multi-core path of
    bass2jax.run_bass_via_pjrt, kept alive so repeat runs skip re-tracing)."""
    import jax
    import numpy as np
    from jax.experimental.shard_map import shard_map
    from jax.sharding import Mesh, PartitionSpec

    import concourse.mybir as mybir
    from concourse.bass2jax import (_bass_exec_p, install_neuronx_cc_hook,
                                    partition_id_tensor)

    install_neuronx_cc_hook()
    partition_name = (nc.partition_id_tensor.name
                      if nc.partition_id_tensor else None)
    in_names, out_names, out_avals, zero_outs = [], [], [], []
    for alloc in nc.m.functions[0].allocations:
        if not isinstance(alloc, mybir.MemoryLocationSet):
            continue
        name = alloc.memorylocations[0].name
        if alloc.kind == "ExternalInput":
            if name != partition_name:
                in_names.append(name)
        elif alloc.kind == "ExternalOutput":
            shape = tuple(alloc.tensor_shape)
            dtype = mybir.dt.np(alloc.dtype)
            out_names.append(name)
            out_avals.append(jax.core.ShapedArray(shape, dtype))
            zero_outs.append(np.zeros(shape, dtype))
    n_params = len(in_names)
    n_outs = len(out_avals)
    all_in_names = list(in_names) + list(out_names)
    if partition_name is not None:
        all_in_names.append(partition_name)
    donate = tuple(range(n_params, n_params + n_outs))

    def _body(*args):
        operands = list(args)
        if partition_name is not None:
            operands.append(partition_id_tensor())
        outs = _bass_exec_p.bind(
            *operands,
            out_avals=tuple(out_avals),
            in_names=tuple(all_in_names),
            out_names=tuple(out_names),
            lowering_input_output_aliases=(),
            sim_require_finite=True,
            sim_require_nnan=True,
            nc=nc,
        )
        return tuple(outs)

    devices = jax.devices()[:NCORES]
    mesh = Mesh(np.asarray(devices), ("core",))
    in_specs = (PartitionSpec("core"),) * (n_params + n_outs)
    out_specs = (PartitionSpec("core"),) * len(out_names)
    sharded = jax.jit(
        shard_map(_body, mesh=mesh, in_specs=in_specs, out_specs=out_specs,
                  check_rep=False),
        donate_argnums=donate, keep_unused=True)

    def prep(in_maps):
        concat_in = [
            np.concatenate([np.asarray(in_maps[c][nm]) for c in range(NCORES)],
                           axis=0)
            for nm in in_names]
        concat_zeros = [np.zeros((NCORES * z.shape[0], *z.shape[1:]), z.dtype)
                        for z in zero_outs]
        return concat_in, concat_zeros

    def run_prepped(concat_in, concat_zeros):
        return sharded(*concat_in, *concat_zeros)

    def run(in_maps):
        concat_in, concat_zeros = prep(in_maps)
        out_arrs = run_prepped(concat_in, concat_zeros)
        return [
            {nm: np.asarray(out_arrs[i]).reshape(NCORES, *out_avals[i].shape)[c]
             for i, nm in enumerate(out_names)}
            for c in range(NCORES)]

    run.prep = prep
    run.run_prepped = run_prepped
    run.mesh = mesh
    return run


def get_runner():
    if "runner" not in _cache:
        if "nc" not in _cache:
            _cache["nc"] = _build_program()
        _cache["runner"] = _make_runner(_cache["nc"])
    return _cache["runner"]


def kernel(x_padded, Wqkv, Wout, seq_lengths, window_left, window_right):
    assert int(window_left) == WIN and int(window_right) == 0
    x_padded = np.asarray(x_padded, dtype=np.float32)
    Wqkv = np.asarray(Wqkv, dtype=np.float32)
    Wout = np.asarray(Wout, dtype=np.float32)
    seq_lengths = np.asarray(seq_lengths, dtype=np.int32)

    run = get_runner()
    in_maps = _prep_inputs(x_padded, Wqkv, Wout, seq_lengths)
    results = run(in_maps)

    out = np.empty((B, S, E), dtype=np.float32)
    for b in range(B):
        acc = (results[2 * b]["outT"].astype(np.float32)
               + results[2 * b + 1]["outT"].astype(np.float32))
        out[b] = acc.T

    # fully-masked query rows: window [i-512, i] entirely past seq_len
    Wv = Wqkv[2 * E:3 * E]
    for b in range(B):
        sl = int(seq_lengths[b])
        if sl == 0:
            v_mean = x_padded[b].mean(axis=0) @ Wv.T
            out[b, :, :] = v_mean @ Wout.T
        elif sl + WIN < S:
            v_mean = x_padded[b].mean(axis=0) @ Wv.T
            out[b, sl + WIN:, :] = v_mean @ Wout.T
    return out



# revision 5
# speedup vs baseline: 1.0563x; 1.0563x over previous
"""Sliding-window attention block (B=4, S=2048, E=1024, H=16, D=64,
window_left=512, window_right=0) on 8 Trainium2 NeuronCores.

Sharding: core c handles batch b=c//2 and head group g=c%2 (8 heads each).
Each core computes qkv projection for its heads over the full sequence,
banded attention (256-query stripes, 128-key blocks), and a partial output
projection; the host sums the two head-group partials per batch.

All device dataflow is feature-major (transposed): qkT/attnT/outT are
[features, seq].  Window masking is added into the scores PSUM with
identity-weight matmuls of precomputed -30000 bias tiles.  Key padding
(j >= seq_len) is handled by zeroing V rows and the denominator-ones
column, so padded keys drop out of both numerator and denominator.
Fully-masked query rows (i >= seq_len+512) are fixed up on the host
(reference semantics: uniform attention over all keys).
"""

import numpy as np

B, S, E, H, D = 4, 2048, 1024, 16, 64
NCORES = 8
HPC = H // 2          # heads per core
WIN = 512             # window_left (window_right = 0)
NEG = -30000.0
NQ = 256              # query stripe width
NST = S // NQ         # stripes
SCALE = 1.0 / np.sqrt(np.float32(D))

_cache = {}


def _build_program(repeat=1, ablate=()):
    from contextlib import ExitStack

    import concourse.bass as bass  # noqa: F401
    import concourse.mybir as mybir
    import concourse.tile as tile
    from concourse import bacc

    dt = mybir.dt
    f32, f32r = dt.float32, dt.float32r
    AF = mybir.ActivationFunctionType
    mult = mybir.AluOpType.mult

    nc = bacc.Bacc("TRN2", target_bir_lowering=False, debug=False,
                   num_devices=NCORES)
    bf16 = dt.bfloat16

    xT = nc.dram_tensor("xT", [E, S], bf16, kind="ExternalInput")
    # wqk laid out [partition, k-chunk, out-col] so per-mb column slices
    # stream in 0.25 MB pieces (fast first-matmul arrival)
    wqk = nc.dram_tensor("wqk", [128, 8, 2 * HPC * D], bf16,
                         kind="ExternalInput")
    wv = nc.dram_tensor("wv", [E, HPC * D], bf16, kind="ExternalInput")
    wodt = f32r if "f32out" in ablate else bf16
    wo = nc.dram_tensor("wo", [HPC * D, E], wodt, kind="ExternalInput")
    vmask = nc.dram_tensor("vmask", [128, 16], f32, kind="ExternalInput")
    vone8 = nc.dram_tensor("vone8", [S, HPC], bf16, kind="ExternalInput")
    # mask planes (cols): [0:512) = [m_a | m_d]; [512:768) =
    # [m_b[:,128:] | m_c[:,:128]]; [768:1152) = [m_d | m_c[:,:128]];
    # [1152:1280) = identity
    # f32r to match the score matmuls' weight dtype — a bf16 128-col ident
    # would toggle the compiler's FWL weight-load mode on every masked bank
    masks = nc.dram_tensor("masks", [128, 1280], f32r, kind="ExternalInput")
    outT = nc.dram_tensor("outT", [E, S], bf16, kind="ExternalOutput")

    with tile.TileContext(nc) as tc:
      for _rep in range(repeat):
       with ExitStack() as ctx:
        persist = ctx.enter_context(tc.tile_pool(name="persist", bufs=1))

        qdt = f32r if "f32qk" in ablate else bf16
        qkT = [persist.tile([128, S], qdt, name=f"qkT{i}", tag=f"qkT{i}") for i in range(8)]
        vsb = [persist.tile([128, HPC, D + 1], bf16, name=f"v{t}", tag=f"v{t}")
               for t in range(16)]
        mskT = persist.tile([128, 1280], f32r, tag="mskT")
        ident = mskT[:, 1152:1280]
        vmsb = persist.tile([128, 16], f32, tag="vmsb")
        adt = f32r if "f32out" in ablate else bf16
        attnT = [persist.tile([128, S], adt, name=f"attnT{i}",
                              tag=f"attnT{i}") for i in range(4)]
        wo_sb = [persist.tile([128, E], adt, name=f"wo{c}", tag=f"wo{c}")
                 for c in range(4)]

        # ---- fused schedule: projection chunk nb, then attention stripes
        # 2nb / 2nb+1 (whose keys end at tile 4nb+3); the stripe-pair's
        # output projection is interleaved into chunk nb+1's qk loop.
        with tc.tile_pool(name="wgt12", bufs=1) as wpool, \
             tc.tile_pool(name="xc", bufs=3) as xpool, \
             tc.tile_pool(name="qkps", bufs=2, space="PSUM") as qkps, \
             tc.tile_pool(name="vps", bufs=2, space="PSUM") as vps, \
             tc.tile_pool(name="scps", bufs=2, space="PSUM") as spool, \
             tc.tile_pool(name="ops", bufs=1, space="PSUM") as opool, \
             tc.tile_pool(name="oprj", bufs=1, space="PSUM") as ppool, \
             tc.tile_pool(name="expT", bufs=6) as epool, \
             tc.tile_pool(name="ob", bufs=3) as obpool, \
             tc.tile_pool(name="rc", bufs=4) as rpool, \
             tc.tile_pool(name="rb", bufs=4) as rbpool:
            xcs = {}

            def load_chunk(nb):
                tiles = [xpool.tile([128, 512], bf16, name=f"xc{k}",
                                    tag=f"xc{k}") for k in range(8)]
                for k in range(8):
                    nc.sync.dma_start(
                        out=tiles[k],
                        in_=xT[k * 128:(k + 1) * 128,
                               nb * 512:(nb + 1) * 512])
                xcs[nb] = tiles

            wqk_sb = wpool.tile([128, 8, 2 * HPC * D], bf16, name="wqk",
                                tag="wqk")
            if "nowdma" not in ablate:
                for mb in range(2):
                    nc.sync.dma_start(
                        out=wqk_sb[:, :, mb * 128:(mb + 1) * 128],
                        in_=wqk[:, :, mb * 128:(mb + 1) * 128])
            else:
                nc.vector.memset(wqk_sb[:, 0, 0:8], 0.125)
            load_chunk(0)
            if "nowdma" not in ablate:
                for mb in range(2, 8):
                    nc.sync.dma_start(
                        out=wqk_sb[:, :, mb * 128:(mb + 1) * 128],
                        in_=wqk[:, :, mb * 128:(mb + 1) * 128])
            wv_sb = [wpool.tile([128, HPC * D], bf16, name=f"wv{k}",
                                tag=f"wv{k}") for k in range(8)]
            for k in range(8):
                if "nowdma" not in ablate:
                    nc.sync.dma_start(out=wv_sb[k],
                                      in_=wv[k * 128:(k + 1) * 128, :])
                else:
                    nc.vector.memset(wv_sb[k][:, 0:8], 0.125)
            nc.sync.dma_start(out=mskT, in_=masks[:, :])
            nc.sync.dma_start(out=vmsb, in_=vmask[:, :])
            domask = "mask" not in ablate

            def stripe_banks(s):
                # Banks pair key blocks so every mask-bias region is ONE
                # contiguous >=256-wide slice of the flat [128, 512] PSUM
                # bank.  Mask spec = (flat_lo, flat_hi, msk_col).
                if s == 0:
                    kb_banks = [([5, 4], (0, 384, 768))]
                elif s == 1:
                    kb_banks = [([2, 3], None), ([5, 4], (0, 384, 768))]
                else:
                    kb_banks = [([0, 5], (0, 512, 0)),
                                ([1, 4], (128, 384, 512)),
                                ([2, 3], None)]
                return kb_banks

            def emit_scores(s, hg, kb_banks):
                base_kt = 2 * s - 4
                all_exs = {}
                for bi, (bks, mspec) in enumerate(kb_banks):
                    scbs = {h: spool.tile([128, 2 * NQ], f32, name=f"sc{h}",
                                          tag="sc") for h in hg}
                    for li, kb in enumerate(bks):
                        for h in hg:
                            po = (h % 2) * 64
                            ktile = base_kt + kb
                            nc.tensor.matmul(
                                scbs[h][:, li * NQ:(li + 1) * NQ],
                                lhsT=qkT[4 + h // 2][
                                    po:po + 64,
                                    ktile * 128:(ktile + 1) * 128],
                                rhs=qkT[h // 2][po:po + 64,
                                                s * NQ:(s + 1) * NQ],
                                start=(li == 0),
                                stop=(li == len(bks) - 1
                                      and not (domask and mspec
                                               and "pemask" in ablate)))
                    if domask and mspec:
                        lo, hi, mc = mspec
                        for h in hg:
                            if "pemask" in ablate:
                                nc.tensor.matmul(
                                    scbs[h][:, lo:hi],
                                    lhsT=ident[:, :],
                                    rhs=mskT[:, mc:mc + hi - lo],
                                    start=False, stop=True)
                            else:
                                # mask bias via DVE add into PSUM — identity
                                # matmuls stall the PE weight-load pipeline
                                nc.vector.tensor_tensor(
                                    out=scbs[h][:, lo:hi],
                                    in0=scbs[h][:, lo:hi],
                                    in1=mskT[:, mc:mc + hi - lo],
                                    op=mybir.AluOpType.add)
                    for h in hg:
                        exb = epool.tile([128, 2 * NQ], bf16, tag="ex")
                        nc.scalar.activation(exb[:, :], scbs[h][:, :],
                                             AF.Exp)
                        all_exs.setdefault(h, []).append(exb)
                return all_exs

            def emit_av(s, hg, kb_banks, all_exs):
                base_kt = 2 * s - 4
                kb_pos = {kb: (bi, li)
                          for bi, (bks, _) in enumerate(kb_banks)
                          for li, kb in enumerate(bks)}
                kbs = sorted(kb_pos)
                # one PSUM bank holds both heads' AV accumulations; the
                # first matmul's start clears the whole bank, so the second
                # head's group must NOT re-assert start
                ot2 = opool.tile([D + 1, len(hg), NQ], f32, tag="ot")
                for hi, h in enumerate(hg):
                    exs = all_exs[h]
                    ot = ot2[:, hi, :]
                    for i, kb in enumerate(kbs):
                        bi2, li = kb_pos[kb]
                        ktile = base_kt + kb
                        nc.tensor.matmul(
                            ot[:, :],
                            lhsT=vsb[ktile][:, h, :],
                            rhs=exs[bi2][:, li * NQ:(li + 1) * NQ],
                            start=(i == 0 and hi == 0),
                            stop=(i == len(kbs) - 1))
                nhg = len(hg)
                if "norm" not in ablate:
                    # one reciprocal + one broadcast covers the whole group
                    rc = rpool.tile([1, nhg, NQ], f32, tag="rc")
                    nc.vector.reciprocal(rc[:, :, :], ot2[D:D + 1, :, :])
                    rb = rbpool.tile([128, nhg, NQ], f32, tag="rb")
                    nc.gpsimd.partition_broadcast(
                        rb.rearrange("p a q -> p (a q)"),
                        rc.rearrange("p a q -> p (a q)"))
                for hi, h in enumerate(hg):
                    po = (h % 2) * 64
                    ot = ot2[:, hi, :]
                    dst = attnT[h // 2][po:po + 64, s * NQ:(s + 1) * NQ]
                    if "norm" in ablate:
                        nc.vector.tensor_copy(dst, ot[0:D, :])
                    else:
                        # evict unnormalized, freeing the PSUM slot quickly;
                        # scale attnT in place off-path
                        if hi % 2 == 0:
                            nc.scalar.copy(dst, ot[0:D, :])
                        else:
                            nc.vector.tensor_copy(dst, ot[0:D, :])
                        nc.vector.tensor_tensor(
                            out=dst, in0=dst, in1=rb[po:po + 64, hi, :],
                            op=mult)

            def emit_outproj_mb(c0, mb, final=False):
                if final:
                    # projection psum pools are idle by now — rotate through
                    # them so the ob copy never blocks the next group
                    pool, tag = [(ppool, "pp"), (qkps, "qk"),
                                 (vps, "v")][mb % 3]
                    pp = pool.tile([128, 2 * NQ], f32, tag=tag)
                else:
                    pp = ppool.tile([128, 2 * NQ], f32, tag="pp")
                for cb in range(4):
                    nc.tensor.matmul(
                        pp[:, :],
                        lhsT=wo_sb[cb][:, mb * 128:(mb + 1) * 128],
                        rhs=attnT[cb][:, c0:c0 + 2 * NQ],
                        start=(cb == 0), stop=(cb == 3))
                ob = obpool.tile([128, 2 * NQ], bf16, tag="ob")
                nc.vector.tensor_copy(ob[:, :], pp[:, :])
                if "noout" not in ablate:
                    nc.sync.dma_start(
                        out=outT[mb * 128:(mb + 1) * 128, c0:c0 + 2 * NQ],
                        in_=ob[:, :])

            if "nopair" in ablate:
                head_groups = [(h,) for h in range(HPC)]
            else:
                head_groups = [(2 * i, 2 * i + 1) for i in range(HPC // 2)]
            pending = None  # software-pipeline AV one head-group behind
            out_c0 = None   # outproj deferred into the next chunk's qk loop
            for nb in range(4):
                if "noxdma" in ablate:
                    if 0 not in xcs:
                        load_chunk(0)
                    xc = xcs[0]
                elif nb not in xcs:
                    load_chunk(nb)
                    xc = xcs.pop(nb)
                else:
                    xc = xcs.pop(nb)
                for mb in range(8):
                    ps = qkps.tile([128, 512], f32, tag="qk")
                    for k in range(8):
                        nc.tensor.matmul(
                            ps[:, :],
                            lhsT=wqk_sb[:, k, mb * 128:(mb + 1) * 128],
                            rhs=xc[k][:, :],
                            start=(k == 0), stop=(k == 7))
                    nc.scalar.copy(qkT[mb][:, nb * 512:(nb + 1) * 512],
                                   ps[:, :])
                    # batched so the f32r wo weight loads don't toggle FWL
                    # against the bf16 projection weights on every group
                    if out_c0 is not None and mb in (3, 7):
                        for omb in range(mb - 3, mb + 1):
                            emit_outproj_mb(out_c0, omb)
                out_c0 = None
                if nb + 1 < 4 and "noxdma" not in ablate:
                    load_chunk(nb + 1)
                if nb == 0:
                    for c in range(4):
                        nc.sync.dma_start(out=wo_sb[c],
                                          in_=wo[c * 128:(c + 1) * 128, :])
                for t4 in range(4):
                    t = nb * 4 + t4
                    ps = vps.tile([128, 512], f32, tag="v")
                    for k in range(8):
                        nc.tensor.matmul(
                            ps[:, :],
                            lhsT=xc[k][:, t4 * 128:(t4 + 1) * 128],
                            rhs=wv_sb[k][:, :],
                            start=(k == 0), stop=(k == 7))
                    nc.vector.tensor_scalar(
                        out=vsb[t][:, :, 0:D],
                        in0=ps.rearrange("p (h d) -> p h d", h=HPC),
                        scalar1=vmsb[:, t:t + 1],
                        scalar2=None,
                        op0=mult)
                    nc.sync.dma_start(out=vsb[t][:, :, D],
                                      in_=vone8[t * 128:(t + 1) * 128, :])
                for s in (2 * nb, 2 * nb + 1):
                    kb_banks = stripe_banks(s)
                    for hg in head_groups:
                        all_exs = emit_scores(s, hg, kb_banks)
                        if pending is not None:
                            emit_av(*pending)
                        pending = (s, hg, kb_banks, all_exs)
                    if s % 2 == 1:
                        if pending is not None:
                            emit_av(*pending)
                            pending = None
                        if nb + 1 < 4:
                            out_c0 = (s - 1) * NQ
                        else:
                            for mb in range(8):
                                emit_outproj_mb((s - 1) * NQ, mb, final=True)

    nc.compile()
    return nc


def _prep_inputs(x_padded, Wqkv, Wout, seq_lengths):
    """Per-core input maps."""
    import ml_dtypes
    bf16 = ml_dtypes.bfloat16
    Wq = Wqkv[0:E]
    Wk = Wqkv[E:2 * E]
    Wv = Wqkv[2 * E:3 * E]

    # static window mask tiles (identical for every core)
    p = np.arange(128)[:, None]
    f = np.arange(NQ)[None, :]
    m_a = np.where(f <= p, 0.0, NEG).astype(np.float32)
    m_b = np.where(f <= p + 128, 0.0, NEG).astype(np.float32)
    m_c = np.where(f >= p, 0.0, NEG).astype(np.float32)
    m_d = np.where(f >= p + 128, 0.0, NEG).astype(np.float32)
    masks = np.concatenate([
        m_a, m_d,                      # [0:512)    bank (kb0, kb5)
        m_b[:, 128:], m_c[:, :128],    # [512:768)  bank (kb1, kb4)
        m_d, m_c[:, :128],             # [768:1152) bank (kb5, kb4)
        np.eye(128, dtype=np.float32),  # [1152:1280) identity
    ], axis=1)
    in_maps = []
    for c in range(NCORES):
        b, g = divmod(c, 2)
        hs = np.arange(g * HPC, (g + 1) * HPC)
        rows = (hs[:, None] * D + np.arange(D)[None, :]).reshape(-1)
        wqk_c = np.concatenate([Wq[rows] * SCALE, Wk[rows]], axis=0)
        valid = (np.arange(S) < seq_lengths[b]).astype(np.float32)
        in_maps.append({
            "xT": np.ascontiguousarray(x_padded[b].T).astype(bf16),
            "wqk": np.ascontiguousarray(
                wqk_c.T.reshape(8, 128, 2 * HPC * D).transpose(1, 0, 2)
            ).astype(bf16),
            "wv": np.ascontiguousarray(Wv[rows].T).astype(bf16),
            "wo": np.ascontiguousarray(Wout[:, rows].T).astype(bf16),
            "vmask": np.ascontiguousarray(valid.reshape(16, 128).T),
            "vone8": np.ascontiguousarray(
                np.repeat(valid[:, None], HPC, axis=1)).astype(bf16),
            "masks": masks,
        })
    return in_maps


def _make_runner(nc):
    """Reusable jitted SPMD executor (the multi-core path of
    bass2jax.run_bass_via_pjrt, kept alive so repeat runs skip re-tracing)."""
    import jax
    import numpy as np
    from jax.experimental.shard_map import shard_map
    from jax.sharding import Mesh, PartitionSpec

    import concourse.mybir as mybir
    from concourse.bass2jax import (_bass_exec_p, install_neuronx_cc_hook,
                                    partition_id_tensor)

    install_neuronx_cc_hook()
    partition_name = (nc.partition_id_tensor.name
                      if nc.partition_id_tensor else None)
    in_names, out_names, out_avals, zero_outs = [], [], [], []
    for alloc in nc.m.functions[0].allocations:
        if not isinstance(alloc, mybir.MemoryLocationSet):
            continue
        name = alloc.memorylocations[0].name
        if alloc.kind == "ExternalInput":
            if name != partition_name:
                in_names.append(name)
        elif alloc.kind == "ExternalOutput":
            shape = tuple(alloc.tensor_shape)
            dtype = mybir.dt.np(alloc.dtype)
            out_names.append(name)
            out_avals.append(jax.core.ShapedArray(shape, dtype))
            zero_outs.append(np.zeros(shape, dtype))
    n_params = len(in_names)
    n_outs = len(out_avals)
    all_in_names = list(in_names) + list(out_names)
    if partition_name is not None:
        all_in_names.append(partition_name)
    donate = tuple(range(n_params, n_params + n_outs))

    def _body(*args):
        operands = list(args)
        if partition_name is not None:
            operands.append(partition_id_tensor())
        outs = _bass_exec_p.bind(
            *operands,
            out_avals=tuple(out_avals),
            in_names=tuple(all_in_names),
            out_names=tuple(out_names),
            lowering_input_output_aliases=(),
            sim_require_finite=True,
            sim_require_nnan=True,
            nc=nc,
        )
        return tuple(outs)

    devices = jax.devices()[:NCORES]
    mesh = Mesh(np.asarray(devices), ("core",))
    in_specs = (PartitionSpec("core"),) * (n_params + n_outs)
    out_specs = (PartitionSpec("core"),) * len(out_names)
    sharded = jax.jit(
        shard_map(_body, mesh=mesh, in_specs=in_specs, out_specs=out_specs,
                  check_rep=False),
        donate_argnums=donate, keep_unused=True)

    def prep(in_maps):
        concat_in = [
            np.concatenate([np.asarray(in_maps[c][nm]) for c in range(NCORES)],
                           axis=0)
            for nm in in_names]
        concat_zeros = [np.zeros((NCORES * z.shape[0], *z.shape[1:]), z.dtype)
                        for z in zero_outs]
        return concat_in, concat_zeros

    def run_prepped(concat_in, concat_zeros):
        return sharded(*concat_in, *concat_zeros)

    def run(in_maps):
        concat_in, concat_zeros = prep(in_maps)
        out_arrs = run_prepped(concat_in, concat_zeros)
        return [
            {nm: np.asarray(out_arrs[i]).reshape(NCORES, *out_avals[i].shape)[c]
             for i, nm in enumerate(out_names)}
            for c in range(NCORES)]

    run.prep = prep
    run.run_prepped = run_prepped
    run.mesh = mesh
    return run


def get_runner():
    if "runner" not in _cache:
        if "nc" not in _cache:
            _cache["nc"] = _build_program()
        _cache["runner"] = _make_runner(_cache["nc"])
    return _cache["runner"]


def kernel(x_padded, Wqkv, Wout, seq_lengths, window_left, window_right):
    assert int(window_left) == WIN and int(window_right) == 0
    x_padded = np.asarray(x_padded, dtype=np.float32)
    Wqkv = np.asarray(Wqkv, dtype=np.float32)
    Wout = np.asarray(Wout, dtype=np.float32)
    seq_lengths = np.asarray(seq_lengths, dtype=np.int32)

    run = get_runner()
    in_maps = _prep_inputs(x_padded, Wqkv, Wout, seq_lengths)
    results = run(in_maps)

    out = np.empty((B, S, E), dtype=np.float32)
    for b in range(B):
        acc = (results[2 * b]["outT"].astype(np.float32)
               + results[2 * b + 1]["outT"].astype(np.float32))
        out[b] = acc.T

    # fully-masked query rows: window [i-512, i] entirely past seq_len
    Wv = Wqkv[2 * E:3 * E]
    for b in range(B):
        sl = int(seq_lengths[b])
        if sl == 0:
            v_mean = x_padded[b].mean(axis=0) @ Wv.T
            out[b, :, :] = v_mean @ Wout.T
        elif sl + WIN < S:
            v_mean = x_padded[b].mean(axis=0) @ Wv.T
            out[b, sl + WIN:, :] = v_mean @ Wout.T
    return out



# revision 47
# speedup vs baseline: 1.4020x; 1.3273x over previous
"""Sliding-window attention block (B=4, S=2048, E=1024, H=16, D=64,
window_left=512, window_right=0) on 8 Trainium2 NeuronCores.

Sharding: core c handles batch b=c//2 and head group g=c%2 (8 heads each).
Each core computes qkv projection for its heads over the full sequence,
banded attention (256-query stripes, 128-key blocks), and a partial output
projection; the host sums the two head-group partials per batch.

All device dataflow is feature-major (transposed): qkT/attnT/outT are
[features, seq].  Window masking is added into the scores PSUM with
identity-weight matmuls of precomputed -30000 bias tiles.  Key padding
(j >= seq_len) is handled by zeroing V rows and the denominator-ones
column, so padded keys drop out of both numerator and denominator.
Fully-masked query rows (i >= seq_len+512) are fixed up on the host
(reference semantics: uniform attention over all keys).
"""

import numpy as np

B, S, E, H, D = 4, 2048, 1024, 16, 64
NCORES = 8
HPC = H // 2          # heads per core
WIN = 512             # window_left (window_right = 0)
NEG = -30000.0
NQ = 256              # query stripe width
NST = S // NQ         # stripes
SCALE = 1.0 / np.sqrt(np.float32(D))

_cache = {}
SIM_REQUIRE_FINITE = True  # CPU-interpreter-only checks (sim_check.py)


def _build_program(repeat=1, ablate=()):
    from contextlib import ExitStack

    import concourse.bass as bass  # noqa: F401
    import concourse.mybir as mybir
    import concourse.tile as tile
    from concourse import bacc

    dt = mybir.dt
    f32, f32r = dt.float32, dt.float32r
    AF = mybir.ActivationFunctionType
    mult = mybir.AluOpType.mult

    nc = bacc.Bacc("TRN2", target_bir_lowering=False, debug=False,
                   num_devices=NCORES)
    bf16 = dt.bfloat16

    xT = nc.dram_tensor("xT", [E, S], bf16, kind="ExternalInput")
    # wqk laid out [partition, k-chunk, out-col] so per-mb column slices
    # stream in 0.25 MB pieces (fast first-matmul arrival)
    wqk = nc.dram_tensor("wqk", [128, 8, 2 * HPC * D], bf16,
                         kind="ExternalInput")
    wv = nc.dram_tensor("wv", [E, HPC * D], bf16, kind="ExternalInput")
    wodt = f32r if "f32out" in ablate else bf16
    wo = nc.dram_tensor("wo", [HPC * D, E], wodt, kind="ExternalInput")
    vmask = nc.dram_tensor("vmask", [128, 16], f32, kind="ExternalInput")
    vone8 = nc.dram_tensor("vone8", [S, HPC], bf16, kind="ExternalInput")
    # mask planes (cols): [0:256) = A = [m_a[:,:128] | m_c[:,:128]];
    # [256:512) = B = [m_b[:,128:] | m_c[:,:128]]; [512:768) = Q =
    # [m_c[:,:128] | m_c[:,:128]]; [768:896) = identity
    masks = nc.dram_tensor("masks", [128, 896], bf16, kind="ExternalInput")
    outT = nc.dram_tensor("outT", [E, S], bf16, kind="ExternalOutput")

    with tile.TileContext(nc) as tc:
      for _rep in range(repeat):
       with ExitStack() as ctx:
        persist = ctx.enter_context(tc.tile_pool(name="persist", bufs=1))

        qdt = f32r if "f32qk" in ablate else bf16
        qkT = [persist.tile([128, S], qdt, name=f"qkT{i}", tag=f"qkT{i}") for i in range(8)]
        vsb = [persist.tile([128, HPC, D + 1], bf16, name=f"v{t}", tag=f"v{t}")
               for t in range(16)]
        mskT = persist.tile([128, 896], bf16, tag="mskT")
        ident = mskT[:, 768:896]
        vmsb = persist.tile([128, 16], f32, tag="vmsb")
        adt = f32r if "f32out" in ablate else bf16
        attnT = [persist.tile([128, S], adt, name=f"attnT{i}",
                              tag=f"attnT{i}") for i in range(4)]
        wo_sb = [persist.tile([128, E], adt, name=f"wo{c}", tag=f"wo{c}")
                 for c in range(4)]
        if "noexp" in ablate:
            shared_ex = persist.tile([128, 2 * NQ], bf16, tag="shex")
            nc.vector.memset(shared_ex[:, :], 0.00390625)
        if "noav" in ablate:
            for c in range(4):
                nc.vector.memset(attnT[c][:, :], 0.01)

        # ---- fused schedule: projection chunk nb, then attention stripes
        # 2nb / 2nb+1 (whose keys end at tile 4nb+3); the stripe-pair's
        # output projection is interleaved into chunk nb+1's qk loop.
        # spool=3/vps=1 raced nondeterministically on hardware; stay at 2/2
        nsc = 3 if "spool3" in ablate else 2
        with tc.tile_pool(name="wgt12", bufs=1) as wpool, \
             tc.tile_pool(name="xc", bufs=3) as xpool, \
             tc.tile_pool(name="qkps", bufs=2, space="PSUM") as qkps, \
             tc.tile_pool(name="vps", bufs=4 - nsc, space="PSUM") as vps, \
             tc.tile_pool(name="scps", bufs=nsc, space="PSUM") as spool, \
             tc.tile_pool(name="ops", bufs=1, space="PSUM") as opool, \
             tc.tile_pool(name="oprj", bufs=1, space="PSUM") as ppool, \
             tc.tile_pool(name="expT", bufs=6) as epool, \
             tc.tile_pool(name="ob", bufs=3) as obpool, \
             tc.tile_pool(name="rc", bufs=4) as rpool, \
             tc.tile_pool(name="rb", bufs=4) as rbpool:
            xcs = {}

            def load_chunk(nb):
                tiles = [xpool.tile([128, 512], bf16, name=f"xc{k}",
                                    tag=f"xc{k}") for k in range(8)]
                for k in range(8):
                    nc.sync.dma_start(
                        out=tiles[k],
                        in_=xT[k * 128:(k + 1) * 128,
                               nb * 512:(nb + 1) * 512])
                xcs[nb] = tiles

            wqk_sb = wpool.tile([128, 8, 2 * HPC * D], bf16, name="wqk",
                                tag="wqk")
            if "nowdma" not in ablate:
                for mb in range(2):
                    nc.sync.dma_start(
                        out=wqk_sb[:, :, mb * 128:(mb + 1) * 128],
                        in_=wqk[:, :, mb * 128:(mb + 1) * 128])
            else:
                nc.vector.memset(wqk_sb[:, 0, 0:8], 0.125)
            load_chunk(0)
            if "nowdma" not in ablate:
                for mb in range(2, 8):
                    nc.sync.dma_start(
                        out=wqk_sb[:, :, mb * 128:(mb + 1) * 128],
                        in_=wqk[:, :, mb * 128:(mb + 1) * 128])
            wv_sb = [wpool.tile([128, HPC * D], bf16, name=f"wv{k}",
                                tag=f"wv{k}") for k in range(8)]
            for k in range(8):
                if "nowdma" not in ablate:
                    nc.sync.dma_start(out=wv_sb[k],
                                      in_=wv[k * 128:(k + 1) * 128, :])
                else:
                    nc.vector.memset(wv_sb[k][:, 0:8], 0.125)
            nc.sync.dma_start(out=mskT, in_=masks[:, :])
            nc.sync.dma_start(out=vmsb, in_=vmask[:, :])
            domask = "mask" not in ablate

            # masks plane column offsets (see _prep_inputs)
            PLANE_A, PLANE_B, PLANE_Q = 0, 256, 512

            def stripe_banks(s):
                # Valid-only packing: each bank lists per-head segments
                # (kb, bank_lo, width, q_lo); fully-masked 128-col halves of
                # the edge key blocks are never computed.  Returns
                # (per_head_banks, shared_segs) where shared_segs packs both
                # heads' kb3 into one bank.
                if s == 0:
                    return ([([(5, 0, 128, 128), (4, 128, 256, 0)],
                              (0, 256, PLANE_Q))], None)
                if s == 1:
                    return ([([(2, 0, 256, 0), (3, 256, 256, 0)], None),
                             ([(5, 0, 128, 128), (4, 128, 256, 0)],
                              (0, 256, PLANE_Q))], None)
                # NOTE: packing both heads' kb3 into one shared bank passes
                # CoreSim but is rejected by the device path (one PSUM
                # accumulation group must not mix partition bases)
                return ([([(0, 0, 128, 0), (5, 128, 128, 128),
                           (2, 256, 256, 0)], (0, 256, PLANE_A)),
                         ([(1, 0, 256, 0), (4, 256, 256, 0)],
                          (128, 384, PLANE_B)),
                         ([(3, 0, 256, 0)], None)], None)

            sc_rot = [0]

            def sc_tile():
                # NOTE: rotating score banks through the proj-shared qkps
                # slots races nondeterministically on hardware (missing WAR
                # against the exp reader); keep scores on their own pool
                if "rot" not in ablate:
                    return spool.tile([128, 2 * NQ], f32, name="scb",
                                      tag="sc")
                pool, tag = ((spool, "sc"), (qkps, "qk"))[sc_rot[0] % 2]
                sc_rot[0] += 1
                return pool.tile([128, 2 * NQ], f32, name="scb", tag=tag)

            def emit_scores(s, hg, kb_banks):
                base_kt = 2 * s - 4
                per_head, shared = kb_banks
                av = {h: [] for h in hg}

                def do_bank(segs_by_head, mspec, hs):
                    wid = max(blo + w for segs in segs_by_head
                              for (kb, blo, w, qlo) in segs)
                    if "noscores" in ablate:
                        for segs, h in zip(segs_by_head, hs):
                            for (kb, blo, w, qlo) in segs:
                                av[h].append((kb, shared_ex, blo, w, qlo))
                        return
                    scb = sc_tile()
                    n = sum(len(segs) for segs in segs_by_head)
                    i = 0
                    masked = domask and mspec and "dvemask" not in ablate
                    for segs, h in zip(segs_by_head, hs):
                        po = (h % 2) * 64
                        for (kb, blo, w, qlo) in segs:
                            ktile = base_kt + kb
                            nc.tensor.matmul(
                                scb[:, blo:blo + w],
                                lhsT=qkT[4 + h // 2][
                                    po:po + 64,
                                    ktile * 128:(ktile + 1) * 128],
                                rhs=qkT[h // 2][
                                    po:po + 64,
                                    s * NQ + qlo:s * NQ + qlo + w],
                                start=(i == 0),
                                stop=(i == n - 1 and not masked))
                            i += 1
                    if domask and mspec:
                        lo, hi, mc = mspec
                        if "dvemask" in ablate:
                            nc.vector.tensor_tensor(
                                out=scb[:, lo:hi], in0=scb[:, lo:hi],
                                in1=mskT[:, mc:mc + hi - lo],
                                op=mybir.AluOpType.add)
                        else:
                            # mask bias via identity matmul on the PE,
                            # which has slack; DVE is the scarce engine
                            nc.tensor.matmul(
                                scb[:, lo:hi], lhsT=ident[:, :],
                                rhs=mskT[:, mc:mc + hi - lo],
                                start=False, stop=True)
                    if "noexp" in ablate:
                        for segs, h in zip(segs_by_head, hs):
                            for (kb, blo, w, qlo) in segs:
                                av[h].append((kb, shared_ex, blo, w, qlo))
                        return
                    exb = epool.tile([128, 2 * NQ], bf16, tag="ex")
                    nc.scalar.activation(exb[:, 0:wid], scb[:, 0:wid],
                                         AF.Exp)
                    for segs, h in zip(segs_by_head, hs):
                        for (kb, blo, w, qlo) in segs:
                            av[h].append((kb, exb, blo, w, qlo))

                # bank-major, head-minor: adjacent matmul blocks alternate
                # PE row-tiles (po 0 vs 64) so the 64-contraction score
                # matmuls of the two heads overlap in the array
                for segs, mspec in per_head:
                    for h in hg:
                        do_bank([segs], mspec, [h])
                if shared is not None:
                    do_bank(shared, None, list(hg))
                return av

            av_rot = [0]

            def av_tile(hg):
                pool, tag = ((opool, "ot"), (ppool, "pp"))[av_rot[0] % 2]
                av_rot[0] += 1
                return pool.tile([D + 1, len(hg), NQ], f32, name="ot2",
                                 tag=tag)

            def emit_av(s, hg, kb_banks, av):
                if "noav" in ablate:
                    return
                base_kt = 2 * s - 4
                # one PSUM bank holds both heads' AV accumulations; the
                # first matmul's start clears the whole bank, so later
                # (partial-width) matmuls must NOT re-assert start
                ot2 = av_tile(hg)
                for hi, h in enumerate(hg):
                    # full-width contributions first: a partial-width matmul
                    # must never be the first writer of a PSUM byte range it
                    # only partially covers (pending-zero granularity)
                    contribs = sorted(av[h], key=lambda t: (t[3] != NQ
                                                            or t[4] != 0,
                                                            t[0]))
                    ot = ot2[:, hi, :]
                    for j, (kb, exb, blo, w, qlo) in enumerate(contribs):
                        nc.tensor.matmul(
                            ot[:, qlo:qlo + w],
                            lhsT=vsb[base_kt + kb][:, h, :],
                            rhs=exb[:, blo:blo + w],
                            start=(j == 0 and hi == 0),
                            stop=(j == len(contribs) - 1
                                  and hi == len(hg) - 1))
                nhg = len(hg)
                if "norm" not in ablate:
                    # one reciprocal + one broadcast covers the whole group
                    rc = rpool.tile([1, nhg, NQ], f32, tag="rc")
                    if "slowrecip" in ablate:
                        nc.vector.reciprocal(rc[:, :, :], ot2[D:D + 1, :, :])
                    else:
                        # approx-fast is a custom bit-trick DVE op; give it a
                        # flat SBUF operand (PSUM src returned garbage)
                        rc0 = rpool.tile([1, nhg * NQ], f32, tag="rc0")
                        nc.scalar.copy(
                            rc0[:, :],
                            ot2[D:D + 1, :, :].rearrange("p a q -> p (a q)"))
                        nc.vector.reciprocal_approx_fast(
                            rc.rearrange("p a q -> p (a q)"), rc0[:, :])
                    rb = rbpool.tile([128, nhg, NQ], f32, tag="rb")
                    nc.gpsimd.partition_broadcast(
                        rb.rearrange("p a q -> p (a q)"),
                        rc.rearrange("p a q -> p (a q)"))
                for hi, h in enumerate(hg):
                    po = (h % 2) * 64
                    ot = ot2[:, hi, :]
                    dst = attnT[h // 2][po:po + 64, s * NQ:(s + 1) * NQ]
                    if "norm" in ablate:
                        nc.vector.tensor_copy(dst, ot[0:D, :])
                    else:
                        # fused evict+normalize straight out of PSUM
                        nc.vector.tensor_tensor(
                            out=dst, in0=ot[0:D, :],
                            in1=rb[po:po + 64, hi, :], op=mult)

            def emit_outproj_mb(c0, mb, pool_tag):
                if "nooutproj" in ablate:
                    return
                pool, tag = pool_tag
                pp = pool.tile([128, 2 * NQ], f32, tag=tag, name="pp")
                for cb in range(4):
                    nc.tensor.matmul(
                        pp[:, :],
                        lhsT=wo_sb[cb][:, mb * 128:(mb + 1) * 128],
                        rhs=attnT[cb][:, c0:c0 + 2 * NQ],
                        start=(cb == 0), stop=(cb == 3))
                ob = obpool.tile([128, 2 * NQ], bf16, tag="ob")
                nc.vector.tensor_copy(ob[:, :], pp[:, :])
                if "noout" not in ablate:
                    nc.sync.dma_start(
                        out=outT[mb * 128:(mb + 1) * 128, c0:c0 + 2 * NQ],
                        in_=ob[:, :])

            def proj_qk_unit(nb, mb, xc):
                ps = qkps.tile([128, 512], f32, tag="qk", name="ps")
                for k in range(8):
                    nc.tensor.matmul(
                        ps[:, :],
                        lhsT=wqk_sb[:, k, mb * 128:(mb + 1) * 128],
                        rhs=xc[k][:, :],
                        start=(k == 0), stop=(k == 7))
                nc.scalar.copy(qkT[mb][:, nb * 512:(nb + 1) * 512], ps[:, :])

            def proj_v_unit(nb, t4, xc):
                t = nb * 4 + t4
                ps = vps.tile([128, 512], f32, tag="v", name="ps")
                for k in range(8):
                    nc.tensor.matmul(
                        ps[:, :],
                        lhsT=xc[k][:, t4 * 128:(t4 + 1) * 128],
                        rhs=wv_sb[k][:, :],
                        start=(k == 0), stop=(k == 7))
                # NOTE: scalar.activation(Copy, scale=AP) matches the sim but
                # mis-executes on hardware — keep this on DVE
                nc.vector.tensor_scalar(
                    out=vsb[t][:, :, 0:D],
                    in0=ps.rearrange("p (h d) -> p h d", h=HPC),
                    scalar1=vmsb[:, t:t + 1],
                    scalar2=None,
                    op0=mult)
                nc.sync.dma_start(out=vsb[t][:, :, D],
                                  in_=vone8[t * 128:(t + 1) * 128, :])

            def get_chunk(n):
                if "noxdma" in ablate:
                    n = 0
                if n not in xcs:
                    load_chunk(n)
                return xcs[n]

            head_groups = [(2 * i, 2 * i + 1) for i in range(HPC // 2)]

            # ---- prologue: project chunk 0 (nothing to overlap with yet)
            xc0 = get_chunk(0)
            for mb in range(8):
                proj_qk_unit(0, mb, xc0)
            for c in range(4):
                nc.sync.dma_start(out=wo_sb[c],
                                  in_=wo[c * 128:(c + 1) * 128, :])
            for t4 in range(4):
                proj_v_unit(0, t4, xc0)

            # ---- steady state (proven fastest): proj chunk nb, then its
            # stripes; the stripe-pair's outproj is batched into the next
            # chunk's qk loop at mb 3/7 so wo weight loads stay clustered.
            pending = None  # software-pipeline AV one head-group behind
            out_c0 = None   # outproj deferred into the next chunk's qk loop
            out_units = []  # ilvo: outproj spread through the next stripes
            for nb in range(4):
                if nb > 0:
                    xc = get_chunk(nb)
                    for mb in range(8):
                        proj_qk_unit(nb, mb, xc)
                        if out_c0 is not None and mb in (3, 7):
                            for omb in range(mb - 3, mb + 1):
                                emit_outproj_mb(out_c0, omb, (ppool, "pp"))
                    out_c0 = None
                    for t4 in range(4):
                        proj_v_unit(nb, t4, xc)
                if nb + 1 < 4 and "noxdma" not in ablate:
                    load_chunk(nb + 1)
                for s in (2 * nb, 2 * nb + 1):
                    kb_banks = stripe_banks(s)
                    for hg in head_groups:
                        avd = emit_scores(s, hg, kb_banks)
                        if pending is not None:
                            emit_av(*pending)
                        pending = (s, hg, kb_banks, avd)
                        if out_units:
                            emit_outproj_mb(*out_units.pop(0))
                    if s % 2 == 1:
                        if pending is not None:
                            emit_av(*pending)
                            pending = None
                        if nb + 1 < 4:
                            if "ilvo" in ablate:
                                out_units = [
                                    ((s - 1) * NQ, mb, (ppool, "pp"))
                                    for mb in range(8)]
                            else:
                                out_c0 = (s - 1) * NQ
                        else:
                            for mb in range(8):
                                emit_outproj_mb(
                                    (s - 1) * NQ, mb,
                                    [(ppool, "pp"), (qkps, "qk"),
                                     (vps, "v")][mb % 3])

    nc.compile()
    return nc


def _prep_inputs(x_padded, Wqkv, Wout, seq_lengths):
    """Per-core input maps."""
    import ml_dtypes
    bf16 = ml_dtypes.bfloat16
    Wq = Wqkv[0:E]
    Wk = Wqkv[E:2 * E]
    Wv = Wqkv[2 * E:3 * E]

    # static window mask tiles (identical for every core)
    p = np.arange(128)[:, None]
    f = np.arange(NQ)[None, :]
    m_a = np.where(f <= p, 0.0, NEG).astype(np.float32)
    m_b = np.where(f <= p + 128, 0.0, NEG).astype(np.float32)
    m_c = np.where(f >= p, 0.0, NEG).astype(np.float32)
    masks = np.concatenate([
        m_a[:, :128], m_c[:, :128],    # plane A (s>=2 edge blocks)
        m_b[:, 128:], m_c[:, :128],    # plane B (s>=2 kb1/kb4 band)
        m_c[:, :128], m_c[:, :128],    # plane Q (s=0/1 edge blocks)
        np.eye(128, dtype=np.float32),  # identity
    ], axis=1)
    in_maps = []
    for c in range(NCORES):
        b, g = divmod(c, 2)
        hs = np.arange(g * HPC, (g + 1) * HPC)
        rows = (hs[:, None] * D + np.arange(D)[None, :]).reshape(-1)
        wqk_c = np.concatenate([Wq[rows] * SCALE, Wk[rows]], axis=0)
        valid = (np.arange(S) < seq_lengths[b]).astype(np.float32)
        in_maps.append({
            "xT": np.ascontiguousarray(x_padded[b].T).astype(bf16),
            "wqk": np.ascontiguousarray(
                wqk_c.T.reshape(8, 128, 2 * HPC * D).transpose(1, 0, 2)
            ).astype(bf16),
            "wv": np.ascontiguousarray(Wv[rows].T).astype(bf16),
            "wo": np.ascontiguousarray(Wout[:, rows].T).astype(bf16),
            "vmask": np.ascontiguousarray(valid.reshape(16, 128).T),
            "vone8": np.ascontiguousarray(
                np.repeat(valid[:, None], HPC, axis=1)).astype(bf16),
            "masks": masks.astype(bf16),
        })
    return in_maps


def _make_runner(nc):
    """Reusable jitted SPMD executor (the multi-core path of
    bass2jax.run_bass_via_pjrt, kept alive so repeat runs skip re-tracing)."""
    import jax
    import numpy as np
    from jax.experimental.shard_map import shard_map
    from jax.sharding import Mesh, PartitionSpec

    import concourse.mybir as mybir
    from concourse.bass2jax import (_bass_exec_p, install_neuronx_cc_hook,
                                    partition_id_tensor)

    install_neuronx_cc_hook()
    partition_name = (nc.partition_id_tensor.name
                      if nc.partition_id_tensor else None)
    in_names, out_names, out_avals, zero_outs = [], [], [], []
    for alloc in nc.m.functions[0].allocations:
        if not isinstance(alloc, mybir.MemoryLocationSet):
            continue
        name = alloc.memorylocations[0].name
        if alloc.kind == "ExternalInput":
            if name != partition_name:
                in_names.append(name)
        elif alloc.kind == "ExternalOutput":
            shape = tuple(alloc.tensor_shape)
            dtype = mybir.dt.np(alloc.dtype)
            out_names.append(name)
            out_avals.append(jax.core.ShapedArray(shape, dtype))
            zero_outs.append(np.zeros(shape, dtype))
    n_params = len(in_names)
    n_outs = len(out_avals)
    all_in_names = list(in_names) + list(out_names)
    if partition_name is not None:
        all_in_names.append(partition_name)
    donate = tuple(range(n_params, n_params + n_outs))

    def _body(*args):
        operands = list(args)
        if partition_name is not None:
            operands.append(partition_id_tensor())
        outs = _bass_exec_p.bind(
            *operands,
            out_avals=tuple(out_avals),
            in_names=tuple(all_in_names),
            out_names=tuple(out_names),
            lowering_input_output_aliases=(),
            sim_require_finite=SIM_REQUIRE_FINITE,
            sim_require_nnan=SIM_REQUIRE_FINITE,
            nc=nc,
        )
        return tuple(outs)

    devices = jax.devices()[:NCORES]
    mesh = Mesh(np.asarray(devices), ("core",))
    in_specs = (PartitionSpec("core"),) * (n_params + n_outs)
    out_specs = (PartitionSpec("core"),) * len(out_names)
    sharded = jax.jit(
        shard_map(_body, mesh=mesh, in_specs=in_specs, out_specs=out_specs,
                  check_rep=False),
        donate_argnums=donate, keep_unused=True)

    def prep(in_maps):
        concat_in = [
            np.concatenate([np.asarray(in_maps[c][nm]) for c in range(NCORES)],
                           axis=0)
            for nm in in_names]
        concat_zeros = [np.zeros((NCORES * z.shape[0], *z.shape[1:]), z.dtype)
                        for z in zero_outs]
        return concat_in, concat_zeros

    def run_prepped(concat_in, concat_zeros):
        return sharded(*concat_in, *concat_zeros)

    def run(in_maps):
        concat_in, concat_zeros = prep(in_maps)
        out_arrs = run_prepped(concat_in, concat_zeros)
        return [
            {nm: np.asarray(out_arrs[i]).reshape(NCORES, *out_avals[i].shape)[c]
             for i, nm in enumerate(out_names)}
            for c in range(NCORES)]

    run.prep = prep
    run.run_prepped = run_prepped
    run.mesh = mesh
    return run


def get_runner():
    if "runner" not in _cache:
        if "nc" not in _cache:
            _cache["nc"] = _build_program()
        _cache["runner"] = _make_runner(_cache["nc"])
    return _cache["runner"]


def kernel(x_padded, Wqkv, Wout, seq_lengths, window_left, window_right):
    assert int(window_left) == WIN and int(window_right) == 0
    x_padded = np.asarray(x_padded, dtype=np.float32)
    Wqkv = np.asarray(Wqkv, dtype=np.float32)
    Wout = np.asarray(Wout, dtype=np.float32)
    seq_lengths = np.asarray(seq_lengths, dtype=np.int32)

    run = get_runner()
    in_maps = _prep_inputs(x_padded, Wqkv, Wout, seq_lengths)
    results = run(in_maps)

    out = np.empty((B, S, E), dtype=np.float32)
    for b in range(B):
        acc = (results[2 * b]["outT"].astype(np.float32)
               + results[2 * b + 1]["outT"].astype(np.float32))
        out[b] = acc.T

    # fully-masked query rows: window [i-512, i] entirely past seq_len
    Wv = Wqkv[2 * E:3 * E]
    for b in range(B):
        sl = int(seq_lengths[b])
        if sl == 0:
            v_mean = x_padded[b].mean(axis=0) @ Wv.T
            out[b, :, :] = v_mean @ Wout.T
        elif sl + WIN < S:
            v_mean = x_padded[b].mean(axis=0) @ Wv.T
            out[b, sl + WIN:, :] = v_mean @ Wout.T
    return out



# revision 49
# speedup vs baseline: 1.4921x; 1.0643x over previous
"""Sliding-window attention block (B=4, S=2048, E=1024, H=16, D=64,
window_left=512, window_right=0) on 8 Trainium2 NeuronCores.

Sharding: core c handles batch b=c//2 and head group g=c%2 (8 heads each).
Each core computes qkv projection for its heads over the full sequence,
banded attention (256-query stripes, 128-key blocks), and a partial output
projection; the host sums the two head-group partials per batch.

All device dataflow is feature-major (transposed): qkT/attnT/outT are
[features, seq].  Window masking is added into the scores PSUM with
identity-weight matmuls of precomputed -30000 bias tiles.  Key padding
(j >= seq_len) is handled by zeroing V rows and the denominator-ones
column, so padded keys drop out of both numerator and denominator.
Fully-masked query rows (i >= seq_len+512) are fixed up on the host
(reference semantics: uniform attention over all keys).
"""

import numpy as np

B, S, E, H, D = 4, 2048, 1024, 16, 64
NCORES = 8
HPC = H // 2          # heads per core
WIN = 512             # window_left (window_right = 0)
NEG = -30000.0
NQ = 256              # query stripe width
NST = S // NQ         # stripes
SCALE = 1.0 / np.sqrt(np.float32(D))

_cache = {}
SIM_REQUIRE_FINITE = True  # CPU-interpreter-only checks (sim_check.py)


def _build_program(repeat=1, ablate=()):
    from contextlib import ExitStack

    import concourse.bass as bass  # noqa: F401
    import concourse.mybir as mybir
    import concourse.tile as tile
    from concourse import bacc

    dt = mybir.dt
    f32, f32r = dt.float32, dt.float32r
    AF = mybir.ActivationFunctionType
    mult = mybir.AluOpType.mult

    nc = bacc.Bacc("TRN2", target_bir_lowering=False, debug=False,
                   num_devices=NCORES)
    bf16 = dt.bfloat16

    xT = nc.dram_tensor("xT", [E, S], bf16, kind="ExternalInput")
    # wqk laid out [partition, k-chunk, out-col] so per-mb column slices
    # stream in 0.25 MB pieces (fast first-matmul arrival)
    wqk = nc.dram_tensor("wqk", [128, 8, 2 * HPC * D], bf16,
                         kind="ExternalInput")
    wv = nc.dram_tensor("wv", [E, HPC * D], bf16, kind="ExternalInput")
    wodt = f32r if "f32out" in ablate else bf16
    wo = nc.dram_tensor("wo", [HPC * D, E], wodt, kind="ExternalInput")
    vmask = nc.dram_tensor("vmask", [128, 16], f32, kind="ExternalInput")
    vone8 = nc.dram_tensor("vone8", [S, HPC], bf16, kind="ExternalInput")
    # mask planes (cols): [0:256) = A = [m_a[:,:128] | m_c[:,:128]];
    # [256:512) = B = [m_b[:,128:] | m_c[:,:128]]; [512:768) = Q =
    # [m_c[:,:128] | m_c[:,:128]]; [768:896) = identity
    masks = nc.dram_tensor("masks", [128, 896], bf16, kind="ExternalInput")
    outT = nc.dram_tensor("outT", [E, S], bf16, kind="ExternalOutput")

    with tile.TileContext(nc) as tc:
      for _rep in range(repeat):
       with ExitStack() as ctx:
        persist = ctx.enter_context(tc.tile_pool(name="persist", bufs=1))

        qdt = f32r if "f32qk" in ablate else bf16
        qkT = [persist.tile([128, S], qdt, name=f"qkT{i}", tag=f"qkT{i}") for i in range(8)]
        vsb = [persist.tile([128, HPC, D + 1], bf16, name=f"v{t}", tag=f"v{t}")
               for t in range(16)]
        mskT = persist.tile([128, 896], bf16, tag="mskT")
        ident = mskT[:, 768:896]
        vmsb = persist.tile([128, 16], f32, tag="vmsb")
        adt = f32r if "f32out" in ablate else bf16
        attnT = [persist.tile([128, S], adt, name=f"attnT{i}",
                              tag=f"attnT{i}") for i in range(4)]
        wo_sb = [persist.tile([128, E], adt, name=f"wo{c}", tag=f"wo{c}")
                 for c in range(4)]
        if "noexp" in ablate:
            shared_ex = persist.tile([128, 2 * NQ], bf16, tag="shex")
            nc.vector.memset(shared_ex[:, :], 0.00390625)
        if "noav" in ablate:
            for c in range(4):
                nc.vector.memset(attnT[c][:, :], 0.01)

        # ---- fused schedule: projection chunk nb, then attention stripes
        # 2nb / 2nb+1 (whose keys end at tile 4nb+3); the stripe-pair's
        # output projection is interleaved into chunk nb+1's qk loop.
        # spool=3/vps=1 raced nondeterministically on hardware; stay at 2/2
        nsc = 3 if "spool3" in ablate else 2
        with tc.tile_pool(name="wgt12", bufs=1) as wpool, \
             tc.tile_pool(name="xc", bufs=3) as xpool, \
             tc.tile_pool(name="qkps", bufs=2, space="PSUM") as qkps, \
             tc.tile_pool(name="vps", bufs=4 - nsc, space="PSUM") as vps, \
             tc.tile_pool(name="scps", bufs=nsc, space="PSUM") as spool, \
             tc.tile_pool(name="ops", bufs=1, space="PSUM") as opool, \
             tc.tile_pool(name="oprj", bufs=1, space="PSUM") as ppool, \
             tc.tile_pool(name="expT",
                          bufs=9 if "epool9" in ablate else 6) as epool, \
             tc.tile_pool(name="ob", bufs=3) as obpool, \
             tc.tile_pool(name="rc", bufs=4) as rpool, \
             tc.tile_pool(name="rb", bufs=4) as rbpool:
            xcs = {}

            def load_chunk(nb):
                tiles = [xpool.tile([128, 512], bf16, name=f"xc{k}",
                                    tag=f"xc{k}") for k in range(8)]
                for k in range(8):
                    nc.sync.dma_start(
                        out=tiles[k],
                        in_=xT[k * 128:(k + 1) * 128,
                               nb * 512:(nb + 1) * 512])
                xcs[nb] = tiles

            wqk_sb = wpool.tile([128, 8, 2 * HPC * D], bf16, name="wqk",
                                tag="wqk")
            if "nowdma" not in ablate:
                for mb in range(2):
                    nc.sync.dma_start(
                        out=wqk_sb[:, :, mb * 128:(mb + 1) * 128],
                        in_=wqk[:, :, mb * 128:(mb + 1) * 128])
            else:
                nc.vector.memset(wqk_sb[:, 0, 0:8], 0.125)
            load_chunk(0)
            if "nowdma" not in ablate:
                for mb in range(2, 8):
                    nc.sync.dma_start(
                        out=wqk_sb[:, :, mb * 128:(mb + 1) * 128],
                        in_=wqk[:, :, mb * 128:(mb + 1) * 128])
            wv_sb = [wpool.tile([128, HPC * D], bf16, name=f"wv{k}",
                                tag=f"wv{k}") for k in range(8)]
            for k in range(8):
                if "nowdma" not in ablate:
                    nc.sync.dma_start(out=wv_sb[k],
                                      in_=wv[k * 128:(k + 1) * 128, :])
                else:
                    nc.vector.memset(wv_sb[k][:, 0:8], 0.125)
            nc.sync.dma_start(out=mskT, in_=masks[:, :])
            nc.sync.dma_start(out=vmsb, in_=vmask[:, :])
            domask = "mask" not in ablate

            # masks plane column offsets (see _prep_inputs)
            PLANE_A, PLANE_B, PLANE_Q = 0, 256, 512

            def stripe_banks(s):
                # Valid-only packing: each bank lists per-head segments
                # (kb, bank_lo, width, q_lo); fully-masked 128-col halves of
                # the edge key blocks are never computed.  Returns
                # (per_head_banks, shared_segs) where shared_segs packs both
                # heads' kb3 into one bank.
                if s == 0:
                    return ([([(5, 0, 128, 128), (4, 128, 256, 0)],
                              (0, 256, PLANE_Q))], None)
                if s == 1:
                    return ([([(2, 0, 256, 0), (3, 256, 256, 0)], None),
                             ([(5, 0, 128, 128), (4, 128, 256, 0)],
                              (0, 256, PLANE_Q))], None)
                # NOTE: packing both heads' kb3 into one shared bank passes
                # CoreSim but is rejected by the device path (one PSUM
                # accumulation group must not mix partition bases)
                return ([([(0, 0, 128, 0), (5, 128, 128, 128),
                           (2, 256, 256, 0)], (0, 256, PLANE_A)),
                         ([(1, 0, 256, 0), (4, 256, 256, 0)],
                          (128, 384, PLANE_B)),
                         ([(3, 0, 256, 0)], None)], None)

            sc_rot = [0]

            def sc_tile():
                # NOTE: rotating score banks through the proj-shared qkps
                # slots races nondeterministically on hardware (missing WAR
                # against the exp reader); keep scores on their own pool
                if "rot" not in ablate:
                    return spool.tile([128, 2 * NQ], f32, name="scb",
                                      tag="sc")
                pool, tag = ((spool, "sc"), (qkps, "qk"))[sc_rot[0] % 2]
                sc_rot[0] += 1
                return pool.tile([128, 2 * NQ], f32, name="scb", tag=tag)

            def emit_scores(s, hg, kb_banks):
                base_kt = 2 * s - 4
                per_head, shared = kb_banks
                av = {h: [] for h in hg}

                def do_bank(segs_by_head, mspec, hs):
                    wid = max(blo + w for segs in segs_by_head
                              for (kb, blo, w, qlo) in segs)
                    if "noscores" in ablate:
                        for segs, h in zip(segs_by_head, hs):
                            for (kb, blo, w, qlo) in segs:
                                av[h].append((kb, shared_ex, blo, w, qlo))
                        return
                    scb = sc_tile()
                    n = sum(len(segs) for segs in segs_by_head)
                    i = 0
                    masked = domask and mspec and "dvemask" not in ablate
                    for segs, h in zip(segs_by_head, hs):
                        po = (h % 2) * 64
                        for (kb, blo, w, qlo) in segs:
                            ktile = base_kt + kb
                            nc.tensor.matmul(
                                scb[:, blo:blo + w],
                                lhsT=qkT[4 + h // 2][
                                    po:po + 64,
                                    ktile * 128:(ktile + 1) * 128],
                                rhs=qkT[h // 2][
                                    po:po + 64,
                                    s * NQ + qlo:s * NQ + qlo + w],
                                start=(i == 0),
                                stop=(i == n - 1 and not masked))
                            i += 1
                    if domask and mspec:
                        lo, hi, mc = mspec
                        if "dvemask" in ablate:
                            nc.vector.tensor_tensor(
                                out=scb[:, lo:hi], in0=scb[:, lo:hi],
                                in1=mskT[:, mc:mc + hi - lo],
                                op=mybir.AluOpType.add)
                        else:
                            # mask bias via identity matmul on the PE,
                            # which has slack; DVE is the scarce engine
                            nc.tensor.matmul(
                                scb[:, lo:hi], lhsT=ident[:, :],
                                rhs=mskT[:, mc:mc + hi - lo],
                                start=False, stop=True)
                    if "noexp" in ablate:
                        for segs, h in zip(segs_by_head, hs):
                            for (kb, blo, w, qlo) in segs:
                                av[h].append((kb, shared_ex, blo, w, qlo))
                        return
                    exb = epool.tile([128, 2 * NQ], bf16, tag="ex")
                    nc.scalar.activation(exb[:, 0:wid], scb[:, 0:wid],
                                         AF.Exp)
                    for segs, h in zip(segs_by_head, hs):
                        for (kb, blo, w, qlo) in segs:
                            av[h].append((kb, exb, blo, w, qlo))

                # bank-major, head-minor: adjacent matmul blocks alternate
                # PE row-tiles (po 0 vs 64) so the 64-contraction score
                # matmuls of the two heads overlap in the array
                for segs, mspec in per_head:
                    for h in hg:
                        do_bank([segs], mspec, [h])
                if shared is not None:
                    do_bank(shared, None, list(hg))
                return av

            av_rot = [0]

            def av_tile(hg):
                pool, tag = ((opool, "ot"), (ppool, "pp"))[av_rot[0] % 2]
                av_rot[0] += 1
                return pool.tile([D + 1, len(hg), NQ], f32, name="ot2",
                                 tag=tag)

            def emit_av(s, hg, kb_banks, av):
                if "noav" in ablate:
                    return
                base_kt = 2 * s - 4
                # one PSUM bank holds both heads' AV accumulations; the
                # first matmul's start clears the whole bank, so later
                # (partial-width) matmuls must NOT re-assert start
                ot2 = av_tile(hg)
                for hi, h in enumerate(hg):
                    # full-width contributions first: a partial-width matmul
                    # must never be the first writer of a PSUM byte range it
                    # only partially covers (pending-zero granularity)
                    contribs = sorted(av[h], key=lambda t: (t[3] != NQ
                                                            or t[4] != 0,
                                                            t[0]))
                    ot = ot2[:, hi, :]
                    for j, (kb, exb, blo, w, qlo) in enumerate(contribs):
                        nc.tensor.matmul(
                            ot[:, qlo:qlo + w],
                            lhsT=vsb[base_kt + kb][:, h, :],
                            rhs=exb[:, blo:blo + w],
                            start=(j == 0 and hi == 0),
                            stop=(j == len(contribs) - 1
                                  and hi == len(hg) - 1))
                nhg = len(hg)
                if "norm" not in ablate:
                    # one reciprocal + one broadcast covers the whole group
                    rc = rpool.tile([1, nhg, NQ], f32, tag="rc")
                    if "slowrecip" in ablate:
                        nc.vector.reciprocal(rc[:, :, :], ot2[D:D + 1, :, :])
                    else:
                        # approx-fast is a custom bit-trick DVE op; give it a
                        # flat SBUF operand (PSUM src returned garbage)
                        rc0 = rpool.tile([1, nhg * NQ], f32, tag="rc0")
                        if "rc0dve" in ablate:
                            nc.vector.tensor_copy(
                                rc0[:, :],
                                ot2[D:D + 1, :, :].rearrange(
                                    "p a q -> p (a q)"))
                        else:
                            nc.scalar.copy(
                                rc0[:, :],
                                ot2[D:D + 1, :, :].rearrange(
                                    "p a q -> p (a q)"))
                        nc.vector.reciprocal_approx_fast(
                            rc.rearrange("p a q -> p (a q)"), rc0[:, :])
                    rb = rbpool.tile([128, nhg, NQ], f32, tag="rb")
                    nc.gpsimd.partition_broadcast(
                        rb.rearrange("p a q -> p (a q)"),
                        rc.rearrange("p a q -> p (a q)"))
                for hi, h in enumerate(hg):
                    po = (h % 2) * 64
                    ot = ot2[:, hi, :]
                    dst = attnT[h // 2][po:po + 64, s * NQ:(s + 1) * NQ]
                    if "norm" in ablate:
                        nc.vector.tensor_copy(dst, ot[0:D, :])
                    else:
                        # fused evict+normalize straight out of PSUM
                        nc.vector.tensor_tensor(
                            out=dst, in0=ot[0:D, :],
                            in1=rb[po:po + 64, hi, :], op=mult)

            def emit_outproj_mb(c0, mb, pool_tag):
                if "nooutproj" in ablate:
                    return
                pool, tag = pool_tag
                pp = pool.tile([128, 2 * NQ], f32, tag=tag, name="pp")
                for cb in range(4):
                    nc.tensor.matmul(
                        pp[:, :],
                        lhsT=wo_sb[cb][:, mb * 128:(mb + 1) * 128],
                        rhs=attnT[cb][:, c0:c0 + 2 * NQ],
                        start=(cb == 0), stop=(cb == 3))
                ob = obpool.tile([128, 2 * NQ], bf16, tag="ob")
                nc.vector.tensor_copy(ob[:, :], pp[:, :])
                if "noout" not in ablate:
                    nc.sync.dma_start(
                        out=outT[mb * 128:(mb + 1) * 128, c0:c0 + 2 * NQ],
                        in_=ob[:, :])

            def proj_qk_unit(nb, mb, xc):
                ps = qkps.tile([128, 512], f32, tag="qk", name="ps")
                for k in range(8):
                    nc.tensor.matmul(
                        ps[:, :],
                        lhsT=wqk_sb[:, k, mb * 128:(mb + 1) * 128],
                        rhs=xc[k][:, :],
                        start=(k == 0), stop=(k == 7))
                nc.scalar.copy(qkT[mb][:, nb * 512:(nb + 1) * 512], ps[:, :])

            def proj_v_unit(nb, t4, xc):
                t = nb * 4 + t4
                ps = vps.tile([128, 512], f32, tag="v", name="ps")
                for k in range(8):
                    nc.tensor.matmul(
                        ps[:, :],
                        lhsT=xc[k][:, t4 * 128:(t4 + 1) * 128],
                        rhs=wv_sb[k][:, :],
                        start=(k == 0), stop=(k == 7))
                # NOTE: scalar.activation(Copy, scale=AP) matches the sim but
                # mis-executes on hardware — keep this on DVE
                nc.vector.tensor_scalar(
                    out=vsb[t][:, :, 0:D],
                    in0=ps.rearrange("p (h d) -> p h d", h=HPC),
                    scalar1=vmsb[:, t:t + 1],
                    scalar2=None,
                    op0=mult)
                nc.sync.dma_start(out=vsb[t][:, :, D],
                                  in_=vone8[t * 128:(t + 1) * 128, :])

            def get_chunk(n):
                if "noxdma" in ablate:
                    n = 0
                if n not in xcs:
                    load_chunk(n)
                return xcs[n]

            head_groups = [(2 * i, 2 * i + 1) for i in range(HPC // 2)]

            # ---- prologue: project chunk 0 (nothing to overlap with yet)
            xc0 = get_chunk(0)
            for mb in range(8):
                proj_qk_unit(0, mb, xc0)
            for c in range(4):
                nc.sync.dma_start(out=wo_sb[c],
                                  in_=wo[c * 128:(c + 1) * 128, :])
            for t4 in range(4):
                proj_v_unit(0, t4, xc0)

            # ---- steady state (proven fastest): proj chunk nb, then its
            # stripes; the stripe-pair's outproj is batched into the next
            # chunk's qk loop at mb 3/7 so wo weight loads stay clustered.
            pending = None  # software-pipeline AV one head-group behind
            out_c0 = None   # outproj deferred into the next chunk's qk loop
            out_units = []  # ilvo: outproj spread through the next stripes
            for nb in range(4):
                if nb > 0:
                    xc = get_chunk(nb)
                    for mb in range(8):
                        proj_qk_unit(nb, mb, xc)
                        if out_c0 is not None and mb in (3, 7):
                            for omb in range(mb - 3, mb + 1):
                                emit_outproj_mb(out_c0, omb, (ppool, "pp"))
                    out_c0 = None
                    for t4 in range(4):
                        proj_v_unit(nb, t4, xc)
                if nb + 1 < 4 and "noxdma" not in ablate:
                    load_chunk(nb + 1)
                for s in (2 * nb, 2 * nb + 1):
                    kb_banks = stripe_banks(s)
                    for hg in head_groups:
                        avd = emit_scores(s, hg, kb_banks)
                        if pending is not None:
                            emit_av(*pending)
                        pending = (s, hg, kb_banks, avd)
                        if out_units:
                            emit_outproj_mb(*out_units.pop(0))
                    if s % 2 == 1:
                        if pending is not None:
                            emit_av(*pending)
                            pending = None
                        if nb + 1 < 4:
                            if "ilvo" in ablate:
                                out_units = [
                                    ((s - 1) * NQ, mb, (ppool, "pp"))
                                    for mb in range(8)]
                            else:
                                out_c0 = (s - 1) * NQ
                        else:
                            for mb in range(8):
                                emit_outproj_mb(
                                    (s - 1) * NQ, mb,
                                    [(ppool, "pp"), (qkps, "qk"),
                                     (vps, "v")][mb % 3])

    nc.compile()
    return nc


def _prep_inputs(x_padded, Wqkv, Wout, seq_lengths):
    """Per-core input maps."""
    import ml_dtypes
    bf16 = ml_dtypes.bfloat16
    Wq = Wqkv[0:E]
    Wk = Wqkv[E:2 * E]
    Wv = Wqkv[2 * E:3 * E]

    # static window mask tiles (identical for every core)
    p = np.arange(128)[:, None]
    f = np.arange(NQ)[None, :]
    m_a = np.where(f <= p, 0.0, NEG).astype(np.float32)
    m_b = np.where(f <= p + 128, 0.0, NEG).astype(np.float32)
    m_c = np.where(f >= p, 0.0, NEG).astype(np.float32)
    masks = np.concatenate([
        m_a[:, :128], m_c[:, :128],    # plane A (s>=2 edge blocks)
        m_b[:, 128:], m_c[:, :128],    # plane B (s>=2 kb1/kb4 band)
        m_c[:, :128], m_c[:, :128],    # plane Q (s=0/1 edge blocks)
        np.eye(128, dtype=np.float32),  # identity
    ], axis=1)
    in_maps = []
    for c in range(NCORES):
        b, g = divmod(c, 2)
        hs = np.arange(g * HPC, (g + 1) * HPC)
        rows = (hs[:, None] * D + np.arange(D)[None, :]).reshape(-1)
        wqk_c = np.concatenate([Wq[rows] * SCALE, Wk[rows]], axis=0)
        valid = (np.arange(S) < seq_lengths[b]).astype(np.float32)
        in_maps.append({
            "xT": np.ascontiguousarray(x_padded[b].T).astype(bf16),
            "wqk": np.ascontiguousarray(
                wqk_c.T.reshape(8, 128, 2 * HPC * D).transpose(1, 0, 2)
            ).astype(bf16),
            "wv": np.ascontiguousarray(Wv[rows].T).astype(bf16),
            "wo": np.ascontiguousarray(Wout[:, rows].T).astype(bf16),
            "vmask": np.ascontiguousarray(valid.reshape(16, 128).T),
            "vone8": np.ascontiguousarray(
                np.repeat(valid[:, None], HPC, axis=1)).astype(bf16),
            "masks": masks.astype(bf16),
        })
    return in_maps


def _make_runner(nc):
    """Reusable jitted SPMD executor (the multi-core path of
    bass2jax.run_bass_via_pjrt, kept alive so repeat runs skip re-tracing)."""
    import jax
    import numpy as np
    from jax.experimental.shard_map import shard_map
    from jax.sharding import Mesh, PartitionSpec

    import concourse.mybir as mybir
    from concourse.bass2jax import (_bass_exec_p, install_neuronx_cc_hook,
                                    partition_id_tensor)

    install_neuronx_cc_hook()
    partition_name = (nc.partition_id_tensor.name
                      if nc.partition_id_tensor else None)
    in_names, out_names, out_avals, zero_outs = [], [], [], []
    for alloc in nc.m.functions[0].allocations:
        if not isinstance(alloc, mybir.MemoryLocationSet):
            continue
        name = alloc.memorylocations[0].name
        if alloc.kind == "ExternalInput":
            if name != partition_name:
                in_names.append(name)
        elif alloc.kind == "ExternalOutput":
            shape = tuple(alloc.tensor_shape)
            dtype = mybir.dt.np(alloc.dtype)
            out_names.append(name)
            out_avals.append(jax.core.ShapedArray(shape, dtype))
            zero_outs.append(np.zeros(shape, dtype))
    n_params = len(in_names)
    n_outs = len(out_avals)
    all_in_names = list(in_names) + list(out_names)
    if partition_name is not None:
        all_in_names.append(partition_name)
    donate = tuple(range(n_params, n_params + n_outs))

    def _body(*args):
        operands = list(args)
        if partition_name is not None:
            operands.append(partition_id_tensor())
        outs = _bass_exec_p.bind(
            *operands,
            out_avals=tuple(out_avals),
            in_names=tuple(all_in_names),
            out_names=tuple(out_names),
            lowering_input_output_aliases=(),
            sim_require_finite=SIM_REQUIRE_FINITE,
            sim_require_nnan=SIM_REQUIRE_FINITE,
            nc=nc,
        )
        return tuple(outs)

    devices = jax.devices()[:NCORES]
    mesh = Mesh(np.asarray(devices), ("core",))
    in_specs = (PartitionSpec("core"),) * (n_params + n_outs)
    out_specs = (PartitionSpec("core"),) * len(out_names)
    sharded = jax.jit(
        shard_map(_body, mesh=mesh, in_specs=in_specs, out_specs=out_specs,
                  check_rep=False),
        donate_argnums=donate, keep_unused=True)

    def prep(in_maps):
        concat_in = [
            np.concatenate([np.asarray(in_maps[c][nm]) for c in range(NCORES)],
                           axis=0)
            for nm in in_names]
        concat_zeros = [np.zeros((NCORES * z.shape[0], *z.shape[1:]), z.dtype)
                        for z in zero_outs]
        return concat_in, concat_zeros

    def run_prepped(concat_in, concat_zeros):
        return sharded(*concat_in, *concat_zeros)

    def run(in_maps):
        concat_in, concat_zeros = prep(in_maps)
        out_arrs = run_prepped(concat_in, concat_zeros)
        return [
            {nm: np.asarray(out_arrs[i]).reshape(NCORES, *out_avals[i].shape)[c]
             for i, nm in enumerate(out_names)}
            for c in range(NCORES)]

    run.prep = prep
    run.run_prepped = run_prepped
    run.mesh = mesh
    return run


def get_runner():
    if "runner" not in _cache:
        if "nc" not in _cache:
            _cache["nc"] = _build_program()
        _cache["runner"] = _make_runner(_cache["nc"])
    return _cache["runner"]


def kernel(x_padded, Wqkv, Wout, seq_lengths, window_left, window_right):
    assert int(window_left) == WIN and int(window_right) == 0
    x_padded = np.asarray(x_padded, dtype=np.float32)
    Wqkv = np.asarray(Wqkv, dtype=np.float32)
    Wout = np.asarray(Wout, dtype=np.float32)
    seq_lengths = np.asarray(seq_lengths, dtype=np.int32)

    run = get_runner()
    in_maps = _prep_inputs(x_padded, Wqkv, Wout, seq_lengths)
    results = run(in_maps)

    out = np.empty((B, S, E), dtype=np.float32)
    for b in range(B):
        acc = (results[2 * b]["outT"].astype(np.float32)
               + results[2 * b + 1]["outT"].astype(np.float32))
        out[b] = acc.T

    # fully-masked query rows: window [i-512, i] entirely past seq_len
    Wv = Wqkv[2 * E:3 * E]
    for b in range(B):
        sl = int(seq_lengths[b])
        if sl == 0:
            v_mean = x_padded[b].mean(axis=0) @ Wv.T
            out[b, :, :] = v_mean @ Wout.T
        elif sl + WIN < S:
            v_mean = x_padded[b].mean(axis=0) @ Wv.T
            out[b, sl + WIN:, :] = v_mean @ Wout.T
    return out

